# revision 20
# baseline (speedup 1.0000x reference)
"""Multi-head attention (RoPE + causal softmax + out-proj) on 8 TRN2 NeuronCores.

Sharding: core c handles batch b = c // 2 and head-half g = c % 2 (8 of 16
heads). Each core computes q/k/v projections for its heads, RoPE, causal
attention, and a partial transposed output projection
outT = (y_heads @ Wo_part.T).T; the host sums the two partials per batch.

v2 design notes (vs the v1 baseline):
 - q/k (post-rope), v, and exp(scores) are bf16: scores matmuls get FWL
   weight loads, narrow free dims keep full PE rate, SBUF halves.
 - Attention is k-major: sT = k q^T in [k:128, q:512] tiles, with the 4
   heads of a group split across two PSUM pair-tiles (sT_AB, sT_CD) so the
   exp activations (ScalarE, the binding engine here) double-buffer with 2
   PSUM slots instead of 4 banks per slot.
 - exp is causally narrowed: diagonal k-tiles only exponentiate the valid
   q-range; the 128-wide triangle strip is masked with a bf16 tril multiply
   on GPSIMD; attn@v / denominator matmuls use the same narrowed range.
 - attn@v packs two heads per 512-cycle PE window via column tiling
   (M=64 at tile_position (0,0)/(0,64)); the softmax denominator is a 4-way
   col-tiled ones-matmul (M=32 each) producing row-replicated sums, so
   normalization is reciprocal + partition_broadcast + one multiply, with
   no DMA staging.
 - Normalization and the output projection run per q-chunk, interleaved
   with the next chunk's attention to keep the PE warm.
"""

import numpy as np

B, T, C, H = 4, 2048, 1024, 16
DH = C // H  # 64
NCORES = 8
HPC = H // 2  # 8 heads per core
QR = HPC * DH  # 512 rows per q/k/v section
TS = 512  # q-chunk width
NTS = T // TS  # 4
CC = C // 128  # 8 contraction chunks
NKT = T // 128  # 16 k-tiles

_CACHE = {}


def _build_program():
    import concourse.mybir as mybir
    import concourse.tile as tile
    from concourse import bacc

    f32 = mybir.dt.float32
    f32r = mybir.dt.float32r
    bf16 = mybir.dt.bfloat16
    EXP = mybir.ActivationFunctionType.Exp

    nc = bacc.Bacc(trn_type="TRN2")

    xT = nc.dram_tensor("xT", [C, T], bf16, kind="ExternalInput").ap()
    wqkvT = nc.dram_tensor("wqkvT", [C, 3 * QR], bf16, kind="ExternalInput").ap()
    woT = nc.dram_tensor("woT", [QR, C], f32, kind="ExternalInput").ap()
    cosT = nc.dram_tensor("cosT", [128, T], f32, kind="ExternalInput").ap()
    sinT = nc.dram_tensor("sinT", [128, T], f32, kind="ExternalInput").ap()
    maskd = nc.dram_tensor("maskd", [128, 128], bf16, kind="ExternalInput").ap()
    outT = nc.dram_tensor("outT", [C, T], f32, kind="ExternalOutput").ap()

    with tile.TileContext(nc) as tc:
        with tc.tile_pool(name="persist", bufs=1) as pp:
            # rope'd q/k, bf16, projection layout: key (part, grp, half):
            # rows 32*i = x-half of local head 4*grp+i
            qk = {}
            for part in range(2):
                for grp in range(2):
                    for half in range(2):
                        nm = f"qk{part}{grp}{half}"
                        qk[(part, grp, half)] = pp.tile(
                            [128, T], bf16, tag=nm, name=nm
                        )
            # v: [t-chunk 128, 8 heads * 64], bf16
            vt = [
                pp.tile([128, QR], bf16, tag=f"vt{t}", name=f"vt{t}")
                for t in range(NKT)
            ]
            maskt = pp.tile([128, 128], bf16, tag="maskt")
            onesw = pp.tile([128, 32], bf16, tag="onesw")
            ct = pp.tile([128, T], f32, tag="cos")
            st_ = pp.tile([128, T], f32, tag="sin")

            # ---------------- phase A: qkv projection + rope ----------------
            with (
                tc.tile_pool(name="wpool", bufs=1) as wp,
                tc.tile_pool(name="xpool", bufs=9) as xp,
                tc.tile_pool(name="ropetmp", bufs=8) as rt,
                tc.tile_pool(name="psA", bufs=4, space="PSUM") as psA,
            ):
                # first weight chunk + first x chunks + trig tables first,
                # then the remaining weight chunks
                wtiles = [None] * CC
                w = wp.tile([128, 3 * QR], bf16, tag="w0", name="w0")
                nc.sync.dma_start(w[:], wqkvT[0:128, :])
                wtiles[0] = w
                xts0 = []
                for cc in range(CC):
                    xt = xp.tile([128, TS], bf16, tag="xts", name=f"x0{cc}")
                    nc.sync.dma_start(xt[:], xT[128 * cc : 128 * (cc + 1), 0:TS])
                    xts0.append(xt)
                for cc in range(1, CC):
                    w = wp.tile([128, 3 * QR], bf16, tag=f"w{cc}", name=f"w{cc}")
                    nc.sync.dma_start(w[:], wqkvT[128 * cc : 128 * (cc + 1), :])
                    wtiles[cc] = w
                nc.sync.dma_start(ct[:], cosT[:])
                nc.sync.dma_start(st_[:], sinT[:])
                nc.sync.dma_start(maskt[:], maskd[:])
                nc.gpsimd.memset(onesw[:], 1.0)

                for ts in range(NTS):
                    if ts == 0:
                        xts = xts0
                    else:
                        xts = []
                        for cc in range(CC):
                            xt = xp.tile([128, TS], bf16, tag="xts", name=f"x{ts}{cc}")
                            nc.sync.dma_start(
                                xt[:],
                                xT[
                                    128 * cc : 128 * (cc + 1), TS * ts : TS * (ts + 1)
                                ],
                            )
                            xts.append(xt)

                    # q/k projections -> rope (mul on DVE, add/sub on
                    # GPSIMD, writing bf16 into the persistent qk tiles)
                    for part in range(2):  # 0=q, 1=k
                        for grp in range(2):  # local heads 4*grp .. 4*grp+3
                            ptiles = []
                            for half in range(2):  # x1, x2
                                p = psA.tile(
                                    [128, TS], f32, tag="proj",
                                    name=f"p{ts}{part}{grp}{half}",
                                )
                                col0 = QR * part + 256 * grp + 128 * half
                                for cc in range(CC):
                                    nc.tensor.matmul(
                                        p[:],
                                        wtiles[cc][:, col0 : col0 + 128],
                                        xts[cc][:],
                                        start=(cc == 0),
                                        stop=(cc == CC - 1),
                                    )
                                ptiles.append(p)
                            x1p, x2p = ptiles
                            csl = ct[:, TS * ts : TS * (ts + 1)]
                            ssl = st_[:, TS * ts : TS * (ts + 1)]
                            o1 = qk[(part, grp, 0)][:, TS * ts : TS * (ts + 1)]
                            o2 = qk[(part, grp, 1)][:, TS * ts : TS * (ts + 1)]
                            t1 = rt.tile([128, TS], f32, tag="rt", name=f"t1{ts}{part}{grp}")
                            t2 = rt.tile([128, TS], f32, tag="rt", name=f"t2{ts}{part}{grp}")
                            nc.vector.tensor_mul(t1[:], x1p[:], csl)
                            nc.vector.tensor_mul(t2[:], x2p[:], ssl)
                            nc.gpsimd.tensor_sub(o1, t1[:], t2[:])
                            t3 = rt.tile([128, TS], f32, tag="rt", name=f"t3{ts}{part}{grp}")
                            t4 = rt.tile([128, TS], f32, tag="rt", name=f"t4{ts}{part}{grp}")
                            nc.vector.tensor_mul(t3[:], x1p[:], ssl)
                            nc.vector.tensor_mul(t4[:], x2p[:], csl)
                            nc.gpsimd.tensor_add(o2, t3[:], t4[:])

                    # v projection; psum -> bf16 copy on ScalarE (idle here)
                    for tr4 in range(4):
                        t = 4 * ts + tr4
                        p = psA.tile([128, QR], f32, tag="proj", name=f"pv{ts}{tr4}")
                        for cc in range(CC):
                            nc.tensor.matmul(
                                p[:],
                                xts[cc][:, 128 * tr4 : 128 * (tr4 + 1)],
                                wtiles[cc][:, 2 * QR : 3 * QR],
                                start=(cc == 0),
                                stop=(cc == CC - 1),
                            )
                        nc.scalar.copy(vt[t][:], p[:])

            # ---------------- phase B: attention + out-proj ----------------
            with (
                tc.tile_pool(name="wopool", bufs=1) as wop,
                tc.tile_pool(name="epool", bufs=6) as ep,
                tc.tile_pool(name="rcpool", bufs=4) as rp,
                tc.tile_pool(name="bcpool", bufs=4) as bp,
                tc.tile_pool(name="ynpool", bufs=8) as yp,
                tc.tile_pool(name="ostage", bufs=4) as osp,
                tc.tile_pool(name="psS", bufs=2, space="PSUM") as psS,
                tc.tile_pool(name="psY", bufs=2, space="PSUM") as psY,
                tc.tile_pool(name="psD", bufs=1, space="PSUM") as psD,
                tc.tile_pool(name="psW", bufs=1, space="PSUM") as psW,
            ):
                wot = []
                for j in range(4):
                    w = wop.tile([128, C], f32r, tag=f"wo{j}", name=f"wo{j}")
                    nc.sync.dma_start(
                        w[:], woT[128 * j : 128 * (j + 1), :].bitcast(f32r)
                    )
                    wot.append(w)

                def outproj_chunk(qj, co, ynormq, pool=None, tag="op"):
                    """One 128-channel out-projection chunk for q-chunk qj:
                    4 dense matmuls + evacuate + store. Interleaved into the
                    next chunk's attention to keep the PE bursts long."""
                    p = (pool or psW).tile([128, TS], f32, tag=tag, name=f"o{qj}{co}")
                    for j in range(4):
                        nc.tensor.matmul(
                            p[:],
                            wot[j][:, 128 * co : 128 * (co + 1)],
                            ynormq[j][:],
                            start=(j == 0),
                            stop=(j == 3),
                        )
                    o = osp.tile([128, TS], f32, tag="os", name=f"os{qj}{co}")
                    nc.vector.tensor_copy(o[:], p[:])
                    nc.sync.dma_start(
                        outT[128 * co : 128 * (co + 1), TS * qj : TS * (qj + 1)],
                        o[:],
                    )

                prev_ynorm = None
                for qi in range(NTS):
                    q0 = TS * qi
                    nkt = 4 * (qi + 1)
                    # schedule of leftover out-proj chunks from qi-1,
                    # spread across this chunk's k-tile iterations
                    op_sched = {}
                    if prev_ynorm is not None:
                        # pairs of chunks: each insertion adds ~1.7us of
                        # dense matmuls, pushing the PE burst past the
                        # ~3.4us HAM warm-up window
                        stride = max(1, (2 * nkt) // 4)
                        for co in range(8):
                            op_sched.setdefault((co // 2) * stride, []).append(co)
                    opk = 0  # global kt counter across both g4 groups
                    ynorm = {}
                    for g4 in range(2):
                        yTp = [
                            psY.tile([128, TS], f32, tag="yT", name=f"yT{g4}_{qi}_{p}")
                            for p in range(2)
                        ]
                        dn = psD.tile([128, TS], f32, tag="dn", name=f"dn{g4}_{qi}")

                        def attnv_denom(kt, ets):
                            """attn@v + denominator matmuls for k-tile kt
                            (emitted one iteration late: software-pipeline
                            skew so these exp-gated matmuls never block
                            the next k-tile's scores in the PE queue)."""
                            r = kt - 4 * qi
                            off = 128 * r if r >= 0 else 0
                            for pr in range(2):
                                for lh in range(2):
                                    h = 4 * g4 + 2 * pr + lh
                                    nc.tensor.matmul(
                                        yTp[pr][64 * lh : 64 * (lh + 1), off:TS],
                                        vt[kt][:, DH * h : DH * (h + 1)],
                                        ets[pr][:, TS * lh + off : TS * (lh + 1)],
                                        start=(kt == 0),
                                        stop=(kt == nkt - 1),
                                        tile_position=(0, 64 * lh),
                                    )
                            for pr in range(2):
                                for lh in range(2):
                                    j32 = 32 * (2 * pr + lh)
                                    nc.tensor.matmul(
                                        dn[j32 : j32 + 32, off:TS],
                                        onesw[:, 0:32],
                                        ets[pr][:, TS * lh + off : TS * (lh + 1)],
                                        start=(kt == 0),
                                        stop=(kt == nkt - 1),
                                        tile_position=(0, j32),
                                    )

                        prev = None
                        for kt in range(nkt):
                            k0 = 128 * kt
                            r = kt - 4 * qi
                            off = 128 * r if r >= 0 else 0
                            sp = [
                                psS.tile(
                                    [128, 2 * TS], f32, tag="sT",
                                    name=f"sT{g4}_{qi}_{kt}_{p}",
                                )
                                for p in range(2)
                            ]
                            # scores: 4 heads on 4 row groups, 2 halves
                            # accumulating per head; half-outer emission so
                            # the 4 start-matmuls run concurrently and the
                            # accumulating round pipelines behind them.
                            for half in range(2):
                                for pr in range(2):
                                    for lh in range(2):
                                        rb = 32 * (2 * pr + lh)
                                        nc.tensor.matmul(
                                            sp[pr][:, TS * lh + off : TS * (lh + 1)],
                                            qk[(1, g4, half)][
                                                rb : rb + 32, k0 : k0 + 128
                                            ],
                                            qk[(0, g4, half)][
                                                rb : rb + 32, q0 + off : q0 + TS
                                            ],
                                            start=(half == 0),
                                            stop=(half == 1),
                                            tile_position=(rb, 0),
                                        )
                            ets = []
                            for pr in range(2):
                                eT = ep.tile(
                                    [128, 2 * TS], bf16, tag="eT",
                                    name=f"eT{g4}_{qi}_{kt}_{pr}",
                                )
                                if off == 0:
                                    nc.scalar.activation(
                                        eT[:], sp[pr][:], EXP, scale=0.125
                                    )
                                else:
                                    e3 = eT.rearrange("p (h q) -> p h q", h=2)
                                    s3 = sp[pr].rearrange("p (h q) -> p h q", h=2)
                                    nc.scalar.activation(
                                        e3[:, :, off:TS],
                                        s3[:, :, off:TS],
                                        EXP,
                                        scale=0.125,
                                    )
                                if r >= 0:
                                    # mask the 128-wide triangle strip
                                    e3 = eT.rearrange("p (h q) -> p h q", h=2)
                                    nc.vector.tensor_mul(
                                        e3[:, :, off : off + 128],
                                        e3[:, :, off : off + 128],
                                        maskt[:].unsqueeze(1).broadcast_to(
                                            [128, 2, 128]
                                        ),
                                    )
                                ets.append(eT)
                            if prev is not None:
                                attnv_denom(*prev)
                            for co in op_sched.get(opk, ()):
                                outproj_chunk(qi - 1, co, prev_ynorm)
                            opk += 1
                            prev = (kt, ets)
                        attnv_denom(*prev)
                        # normalization: evacuate yT psum -> sbuf (frees
                        # psY fast), fast-approx reciprocal, stage each
                        # head's denominator row to partition 0 (DMA),
                        # partition_broadcast, multiply.
                        ysb = []
                        for pr in range(2):
                            ys = rp.tile(
                                [128, TS], f32, tag="ysb", name=f"ys{g4}_{qi}_{pr}"
                            )
                            nc.vector.tensor_copy(ys[:], yTp[pr][:])
                            ysb.append(ys)
                        rcp = rp.tile([128, TS], f32, tag="rcp", name=f"rcp{g4}_{qi}")
                        nc.vector.reciprocal_approx_fast(rcp[:], dn[:])
                        for pr in range(2):
                            yn = yp.tile(
                                [128, TS], f32r, tag="yn", name=f"yn{g4}_{qi}_{pr}"
                            )
                            for lh in range(2):
                                j32 = 32 * (2 * pr + lh)
                                stg = rp.tile(
                                    [1, TS], f32, tag="stg", name=f"sg{g4}{qi}{pr}{lh}"
                                )
                                nc.sync.dma_start(stg[:], rcp[j32 : j32 + 1, :])
                                bc = bp.tile(
                                    [128, TS], f32, tag="bc", name=f"bc{g4}{qi}{pr}{lh}"
                                )
                                nc.gpsimd.partition_broadcast(bc[:], stg[:])
                                nc.vector.tensor_mul(
                                    yn[64 * lh : 64 * (lh + 1), :],
                                    ysb[pr][64 * lh : 64 * (lh + 1), :],
                                    bc[64 * lh : 64 * (lh + 1), :],
                                )
                            ynorm[2 * g4 + pr] = yn
                    # any leftover chunks of qi-1 not yet emitted
                    if prev_ynorm is not None:
                        for k2 in sorted(op_sched):
                            if k2 >= opk:
                                for co in op_sched[k2]:
                                    outproj_chunk(qi - 1, co, prev_ynorm)
                    prev_ynorm = ynorm
                # final q-chunk's out-projection (pipeline tail):
                # rotate across the now-idle PSUM pools for 4-deep overlap
                tail_pools = [(psW, "op"), (psD, "dn"), (psY, "yT"), (psY, "yT")]
                for co in range(8):
                    pool, tag = tail_pools[co % 4]
                    outproj_chunk(NTS - 1, co, prev_ynorm, pool=pool, tag=tag)

    nc.compile()
    return nc


def _get_program():
    if "nc" not in _CACHE:
        _CACHE["nc"] = _build_program()
    return _CACHE["nc"]


def _host_inputs(x, cos, sin, Wqkv, Wo):
    """Build the 8 per-core input maps."""
    import ml_dtypes

    # permutation of one head-section's 512 rows (head-relative):
    # row-tile layout [x1 h0-3 | x2 h0-3 | x1 h4-7 | x2 h4-7], 32 rows/block
    perm = []
    for grp in range(2):
        for half in range(2):
            for lh in range(4 * grp, 4 * grp + 4):
                for jj in range(32):
                    perm.append(64 * lh + 2 * jj + half)
    perm = np.asarray(perm)

    cosT4 = np.ascontiguousarray(np.tile(cos.T, (4, 1)).astype(np.float32))
    sinT4 = np.ascontiguousarray(np.tile(sin.T, (4, 1)).astype(np.float32))

    # tril mask block [128, 128]: m[i, j] = 1 iff j >= i, bf16
    tri = (np.arange(128)[:, None] <= np.arange(128)[None, :]).astype(
        ml_dtypes.bfloat16
    )
    tri = np.ascontiguousarray(tri)

    in_maps = []
    for c in range(NCORES):
        b, g = c // 2, c % 2
        hs0 = HPC * g
        sec = np.arange(QR) + DH * hs0  # this core's rows within a section
        Wq = Wqkv[sec[perm], :]
        Wk = Wqkv[C + sec[perm], :]
        Wv = Wqkv[2 * C + sec, :]
        wqkvT = np.ascontiguousarray(
            np.concatenate([Wq, Wk, Wv], 0).T.astype(ml_dtypes.bfloat16)
        )
        woTc = np.ascontiguousarray(Wo[:, sec].T)
        xTb = np.ascontiguousarray(x[b].T.astype(ml_dtypes.bfloat16))
        in_maps.append(
            {
                "xT": xTb,
                "wqkvT": wqkvT,
                "woT": woTc,
                "cosT": cosT4,
                "sinT": sinT4,
                "maskd": tri,
            }
        )
    return in_maps


def kernel(x, cos, sin, Wqkv, Wo, _want_profile=False):
    from concourse.bass_utils import run_bass_kernel_spmd

    x = np.asarray(x, dtype=np.float32)
    cos = np.asarray(cos, dtype=np.float32)
    sin = np.asarray(sin, dtype=np.float32)
    Wqkv = np.asarray(Wqkv, dtype=np.float32)
    Wo = np.asarray(Wo, dtype=np.float32)

    nc = _get_program()
    in_maps = _host_inputs(x, cos, sin, Wqkv, Wo)
    res = run_bass_kernel_spmd(nc, in_maps, list(range(NCORES)), trace=_want_profile)
    out = np.empty((B, T, C), dtype=np.float32)
    for b in range(B):
        acc = (
            res.results[2 * b]["outT"].astype(np.float32)
            + res.results[2 * b + 1]["outT"].astype(np.float32)
        )
        out[b] = acc.T
    if _want_profile:
        return out, res
    return out


# revision 21
# speedup vs baseline: 1.0468x; 1.0468x over previous
"""Multi-head attention (RoPE + causal softmax + out-proj) on 8 TRN2 NeuronCores.

Sharding: core c handles batch b = c // 2 and head-half g = c % 2 (8 of 16
heads). Each core computes q/k/v projections for its heads, RoPE, causal
attention, and a partial transposed output projection
outT = (y_heads @ Wo_part.T).T; the host sums the two partials per batch.

v2 design notes (vs the v1 baseline):
 - q/k (post-rope), v, and exp(scores) are bf16: scores matmuls get FWL
   weight loads, narrow free dims keep full PE rate, SBUF halves.
 - Attention is k-major: sT = k q^T in [k:128, q:512] tiles, with the 4
   heads of a group split across two PSUM pair-tiles (sT_AB, sT_CD) so the
   exp activations (ScalarE, the binding engine here) double-buffer with 2
   PSUM slots instead of 4 banks per slot.
 - exp is causally narrowed: diagonal k-tiles only exponentiate the valid
   q-range; the 128-wide triangle strip is masked with a bf16 tril multiply
   on GPSIMD; attn@v / denominator matmuls use the same narrowed range.
 - attn@v packs two heads per 512-cycle PE window via column tiling
   (M=64 at tile_position (0,0)/(0,64)); the softmax denominator is a 4-way
   col-tiled ones-matmul (M=32 each) producing row-replicated sums, so
   normalization is reciprocal + partition_broadcast + one multiply, with
   no DMA staging.
 - Normalization and the output projection run per q-chunk, interleaved
   with the next chunk's attention to keep the PE warm.
"""

import numpy as np

B, T, C, H = 4, 2048, 1024, 16
DH = C // H  # 64
NCORES = 8
HPC = H // 2  # 8 heads per core
QR = HPC * DH  # 512 rows per q/k/v section
TS = 512  # q-chunk width
NTS = T // TS  # 4
CC = C // 128  # 8 contraction chunks
NKT = T // 128  # 16 k-tiles

_CACHE = {}


def _build_program():
    import concourse.mybir as mybir
    import concourse.tile as tile
    from concourse import bacc

    f32 = mybir.dt.float32
    f32r = mybir.dt.float32r
    bf16 = mybir.dt.bfloat16
    EXP = mybir.ActivationFunctionType.Exp

    nc = bacc.Bacc(trn_type="TRN2")

    xT = nc.dram_tensor("xT", [C, T], bf16, kind="ExternalInput").ap()
    wqkvT = nc.dram_tensor("wqkvT", [C, 3 * QR], bf16, kind="ExternalInput").ap()
    woT = nc.dram_tensor("woT", [QR, C], f32, kind="ExternalInput").ap()
    cosT = nc.dram_tensor("cosT", [128, T], f32, kind="ExternalInput").ap()
    sinT = nc.dram_tensor("sinT", [128, T], f32, kind="ExternalInput").ap()
    maskd = nc.dram_tensor("maskd", [128, 128], bf16, kind="ExternalInput").ap()
    outT = nc.dram_tensor("outT", [C, T], f32, kind="ExternalOutput").ap()

    with tile.TileContext(nc) as tc:
        with tc.tile_pool(name="persist", bufs=1) as pp:
            # rope'd q/k, bf16, projection layout: key (part, grp, half):
            # rows 32*i = x-half of local head 4*grp+i
            qk = {}
            for part in range(2):
                for grp in range(2):
                    for half in range(2):
                        nm = f"qk{part}{grp}{half}"
                        qk[(part, grp, half)] = pp.tile(
                            [128, T], bf16, tag=nm, name=nm
                        )
            # v: [t-chunk 128, 8 heads * 64], bf16
            vt = [
                pp.tile([128, QR], bf16, tag=f"vt{t}", name=f"vt{t}")
                for t in range(NKT)
            ]
            maskt = pp.tile([128, 128], bf16, tag="maskt")
            onesw = pp.tile([128, 32], bf16, tag="onesw")
            ct = pp.tile([128, T], f32, tag="cos")
            st_ = pp.tile([128, T], f32, tag="sin")

            # ---------------- phase A: qkv projection + rope ----------------
            with (
                tc.tile_pool(name="wpool", bufs=1) as wp,
                tc.tile_pool(name="xpool", bufs=9) as xp,
                tc.tile_pool(name="ropetmp", bufs=8) as rt,
                tc.tile_pool(name="psA", bufs=4, space="PSUM") as psA,
            ):
                # first weight chunk + first x chunks + trig tables first,
                # then the remaining weight chunks
                wtiles = [None] * CC
                w = wp.tile([128, 3 * QR], bf16, tag="w0", name="w0")
                nc.sync.dma_start(w[:], wqkvT[0:128, :])
                wtiles[0] = w
                xts0 = []
                for cc in range(CC):
                    xt = xp.tile([128, TS], bf16, tag="xts", name=f"x0{cc}")
                    nc.sync.dma_start(xt[:], xT[128 * cc : 128 * (cc + 1), 0:TS])
                    xts0.append(xt)
                for cc in range(1, CC):
                    w = wp.tile([128, 3 * QR], bf16, tag=f"w{cc}", name=f"w{cc}")
                    nc.sync.dma_start(w[:], wqkvT[128 * cc : 128 * (cc + 1), :])
                    wtiles[cc] = w
                nc.sync.dma_start(ct[:], cosT[:])
                nc.sync.dma_start(st_[:], sinT[:])
                nc.sync.dma_start(maskt[:], maskd[:])
                nc.gpsimd.memset(onesw[:], 1.0)

                for ts in range(NTS):
                    if ts == 0:
                        xts = xts0
                    else:
                        xts = []
                        for cc in range(CC):
                            xt = xp.tile([128, TS], bf16, tag="xts", name=f"x{ts}{cc}")
                            nc.sync.dma_start(
                                xt[:],
                                xT[
                                    128 * cc : 128 * (cc + 1), TS * ts : TS * (ts + 1)
                                ],
                            )
                            xts.append(xt)

                    # q/k projections -> rope (mul on DVE, add/sub on
                    # GPSIMD, writing bf16 into the persistent qk tiles)
                    for part in range(2):  # 0=q, 1=k
                        for grp in range(2):  # local heads 4*grp .. 4*grp+3
                            ptiles = []
                            for half in range(2):  # x1, x2
                                p = psA.tile(
                                    [128, TS], f32, tag="proj",
                                    name=f"p{ts}{part}{grp}{half}",
                                )
                                col0 = QR * part + 256 * grp + 128 * half
                                for cc in range(CC):
                                    nc.tensor.matmul(
                                        p[:],
                                        wtiles[cc][:, col0 : col0 + 128],
                                        xts[cc][:],
                                        start=(cc == 0),
                                        stop=(cc == CC - 1),
                                    )
                                ptiles.append(p)
                            x1p, x2p = ptiles
                            csl = ct[:, TS * ts : TS * (ts + 1)]
                            ssl = st_[:, TS * ts : TS * (ts + 1)]
                            o1 = qk[(part, grp, 0)][:, TS * ts : TS * (ts + 1)]
                            o2 = qk[(part, grp, 1)][:, TS * ts : TS * (ts + 1)]
                            t1 = rt.tile([128, TS], f32, tag="rt", name=f"t1{ts}{part}{grp}")
                            t2 = rt.tile([128, TS], f32, tag="rt", name=f"t2{ts}{part}{grp}")
                            nc.vector.tensor_mul(t1[:], x1p[:], csl)
                            nc.vector.tensor_mul(t2[:], x2p[:], ssl)
                            nc.gpsimd.tensor_sub(o1, t1[:], t2[:])
                            t3 = rt.tile([128, TS], f32, tag="rt", name=f"t3{ts}{part}{grp}")
                            t4 = rt.tile([128, TS], f32, tag="rt", name=f"t4{ts}{part}{grp}")
                            nc.vector.tensor_mul(t3[:], x1p[:], ssl)
                            nc.vector.tensor_mul(t4[:], x2p[:], csl)
                            nc.gpsimd.tensor_add(o2, t3[:], t4[:])

                    # v projection; psum -> bf16 copy on ScalarE (idle here)
                    for tr4 in range(4):
                        t = 4 * ts + tr4
                        p = psA.tile([128, QR], f32, tag="proj", name=f"pv{ts}{tr4}")
                        for cc in range(CC):
                            nc.tensor.matmul(
                                p[:],
                                xts[cc][:, 128 * tr4 : 128 * (tr4 + 1)],
                                wtiles[cc][:, 2 * QR : 3 * QR],
                                start=(cc == 0),
                                stop=(cc == CC - 1),
                            )
                        nc.scalar.copy(vt[t][:], p[:])

            # ---------------- phase B: attention + out-proj ----------------
            with (
                tc.tile_pool(name="wopool", bufs=1) as wop,
                tc.tile_pool(name="epool", bufs=6) as ep,
                tc.tile_pool(name="rcpool", bufs=4) as rp,
                tc.tile_pool(name="bcpool", bufs=4) as bp,
                tc.tile_pool(name="ynpool", bufs=8) as yp,
                tc.tile_pool(name="ostage", bufs=4) as osp,
                tc.tile_pool(name="psS", bufs=2, space="PSUM") as psS,
                tc.tile_pool(name="psY", bufs=2, space="PSUM") as psY,
                tc.tile_pool(name="psD", bufs=1, space="PSUM") as psD,
                tc.tile_pool(name="psW", bufs=1, space="PSUM") as psW,
            ):
                wot = []
                for j in range(4):
                    w = wop.tile([128, C], f32r, tag=f"wo{j}", name=f"wo{j}")
                    nc.sync.dma_start(
                        w[:], woT[128 * j : 128 * (j + 1), :].bitcast(f32r)
                    )
                    wot.append(w)

                def outproj_chunk(qj, co, ynormq, pool=None, tag="op"):
                    """One 128-channel out-projection chunk for q-chunk qj:
                    4 dense matmuls + evacuate + store. Interleaved into the
                    next chunk's attention to keep the PE bursts long."""
                    p = (pool or psW).tile([128, TS], f32, tag=tag, name=f"o{qj}{co}")
                    for j in range(4):
                        nc.tensor.matmul(
                            p[:],
                            wot[j][:, 128 * co : 128 * (co + 1)],
                            ynormq[j][:],
                            start=(j == 0),
                            stop=(j == 3),
                        )
                    o = osp.tile([128, TS], f32, tag="os", name=f"os{qj}{co}")
                    nc.vector.tensor_copy(o[:], p[:])
                    nc.sync.dma_start(
                        outT[128 * co : 128 * (co + 1), TS * qj : TS * (qj + 1)],
                        o[:],
                    )

                prev_ynorm = None
                for qi in range(NTS):
                    q0 = TS * qi
                    nkt = 4 * (qi + 1)
                    # schedule of leftover out-proj chunks from qi-1,
                    # spread across this chunk's k-tile iterations
                    op_sched = {}
                    if prev_ynorm is not None:
                        stride = max(1, (2 * nkt) // 8)
                        for co in range(8):
                            op_sched.setdefault(co * stride, []).append(co)
                    opk = 0  # global kt counter across both g4 groups
                    ynorm = {}
                    for g4 in range(2):
                        yTp = [
                            psY.tile([128, TS], f32, tag="yT", name=f"yT{g4}_{qi}_{p}")
                            for p in range(2)
                        ]
                        dn = psD.tile([128, TS], f32, tag="dn", name=f"dn{g4}_{qi}")

                        def attnv_denom(kt, ets):
                            """attn@v + denominator matmuls for k-tile kt
                            (emitted one iteration late: software-pipeline
                            skew so these exp-gated matmuls never block
                            the next k-tile's scores in the PE queue)."""
                            r = kt - 4 * qi
                            off = 128 * r if r >= 0 else 0
                            for pr in range(2):
                                for lh in range(2):
                                    h = 4 * g4 + 2 * pr + lh
                                    nc.tensor.matmul(
                                        yTp[pr][64 * lh : 64 * (lh + 1), off:TS],
                                        vt[kt][:, DH * h : DH * (h + 1)],
                                        ets[pr][:, TS * lh + off : TS * (lh + 1)],
                                        start=(kt == 0),
                                        stop=(kt == nkt - 1),
                                        tile_position=(0, 64 * lh),
                                    )
                            for pr in range(2):
                                for lh in range(2):
                                    j32 = 32 * (2 * pr + lh)
                                    nc.tensor.matmul(
                                        dn[j32 : j32 + 32, off:TS],
                                        onesw[:, 0:32],
                                        ets[pr][:, TS * lh + off : TS * (lh + 1)],
                                        start=(kt == 0),
                                        stop=(kt == nkt - 1),
                                        tile_position=(0, j32),
                                    )

                        prev = None
                        for kt in range(nkt):
                            k0 = 128 * kt
                            r = kt - 4 * qi
                            off = 128 * r if r >= 0 else 0
                            sp = [
                                psS.tile(
                                    [128, 2 * TS], f32, tag="sT",
                                    name=f"sT{g4}_{qi}_{kt}_{p}",
                                )
                                for p in range(2)
                            ]
                            # scores: 4 heads on 4 row groups, 2 halves
                            # accumulating per head; half-outer emission so
                            # the 4 start-matmuls run concurrently and the
                            # accumulating round pipelines behind them.
                            for half in range(2):
                                for pr in range(2):
                                    for lh in range(2):
                                        rb = 32 * (2 * pr + lh)
                                        nc.tensor.matmul(
                                            sp[pr][:, TS * lh + off : TS * (lh + 1)],
                                            qk[(1, g4, half)][
                                                rb : rb + 32, k0 : k0 + 128
                                            ],
                                            qk[(0, g4, half)][
                                                rb : rb + 32, q0 + off : q0 + TS
                                            ],
                                            start=(half == 0),
                                            stop=(half == 1),
                                            tile_position=(rb, 0),
                                        )
                            ets = []
                            for pr in range(2):
                                eT = ep.tile(
                                    [128, 2 * TS], bf16, tag="eT",
                                    name=f"eT{g4}_{qi}_{kt}_{pr}",
                                )
                                if off == 0:
                                    nc.scalar.activation(
                                        eT[:], sp[pr][:], EXP, scale=0.125
                                    )
                                else:
                                    e3 = eT.rearrange("p (h q) -> p h q", h=2)
                                    s3 = sp[pr].rearrange("p (h q) -> p h q", h=2)
                                    nc.scalar.activation(
                                        e3[:, :, off:TS],
                                        s3[:, :, off:TS],
                                        EXP,
                                        scale=0.125,
                                    )
                                if r >= 0:
                                    # mask the 128-wide triangle strip
                                    e3 = eT.rearrange("p (h q) -> p h q", h=2)
                                    nc.vector.tensor_mul(
                                        e3[:, :, off : off + 128],
                                        e3[:, :, off : off + 128],
                                        maskt[:].unsqueeze(1).broadcast_to(
                                            [128, 2, 128]
                                        ),
                                    )
                                ets.append(eT)
                            if prev is not None:
                                attnv_denom(*prev)
                            for co in op_sched.get(opk, ()):
                                outproj_chunk(qi - 1, co, prev_ynorm)
                            opk += 1
                            prev = (kt, ets)
                        attnv_denom(*prev)
                        # normalization: evacuate yT psum -> sbuf (frees
                        # psY fast), fast-approx reciprocal, stage each
                        # head's denominator row to partition 0 (DMA),
                        # partition_broadcast, multiply.
                        ysb = []
                        for pr in range(2):
                            ys = rp.tile(
                                [128, TS], f32, tag="ysb", name=f"ys{g4}_{qi}_{pr}"
                            )
                            nc.vector.tensor_copy(ys[:], yTp[pr][:])
                            ysb.append(ys)
                        rcp = rp.tile([128, TS], f32, tag="rcp", name=f"rcp{g4}_{qi}")
                        nc.vector.reciprocal_approx_fast(rcp[:], dn[:])
                        for pr in range(2):
                            yn = yp.tile(
                                [128, TS], f32r, tag="yn", name=f"yn{g4}_{qi}_{pr}"
                            )
                            for lh in range(2):
                                j32 = 32 * (2 * pr + lh)
                                stg = rp.tile(
                                    [1, TS], f32, tag="stg", name=f"sg{g4}{qi}{pr}{lh}"
                                )
                                nc.sync.dma_start(stg[:], rcp[j32 : j32 + 1, :])
                                bc = bp.tile(
                                    [128, TS], f32, tag="bc", name=f"bc{g4}{qi}{pr}{lh}"
                                )
                                nc.gpsimd.partition_broadcast(bc[:], stg[:])
                                nc.vector.tensor_mul(
                                    yn[64 * lh : 64 * (lh + 1), :],
                                    ysb[pr][64 * lh : 64 * (lh + 1), :],
                                    bc[64 * lh : 64 * (lh + 1), :],
                                )
                            ynorm[2 * g4 + pr] = yn
                    # any leftover chunks of qi-1 not yet emitted
                    if prev_ynorm is not None:
                        for k2 in sorted(op_sched):
                            if k2 >= opk:
                                for co in op_sched[k2]:
                                    outproj_chunk(qi - 1, co, prev_ynorm)
                    prev_ynorm = ynorm
                # final q-chunk's out-projection (pipeline tail):
                # rotate across the now-idle PSUM pools for 4-deep overlap
                tail_pools = [(psW, "op"), (psD, "dn"), (psY, "yT"), (psY, "yT")]
                for co in range(8):
                    pool, tag = tail_pools[co % 4]
                    outproj_chunk(NTS - 1, co, prev_ynorm, pool=pool, tag=tag)

    nc.compile()
    return nc


def _get_program():
    if "nc" not in _CACHE:
        _CACHE["nc"] = _build_program()
    return _CACHE["nc"]


def _host_inputs(x, cos, sin, Wqkv, Wo):
    """Build the 8 per-core input maps."""
    import ml_dtypes

    # permutation of one head-section's 512 rows (head-relative):
    # row-tile layout [x1 h0-3 | x2 h0-3 | x1 h4-7 | x2 h4-7], 32 rows/block
    perm = []
    for grp in range(2):
        for half in range(2):
            for lh in range(4 * grp, 4 * grp + 4):
                for jj in range(32):
                    perm.append(64 * lh + 2 * jj + half)
    perm = np.asarray(perm)

    cosT4 = np.ascontiguousarray(np.tile(cos.T, (4, 1)).astype(np.float32))
    sinT4 = np.ascontiguousarray(np.tile(sin.T, (4, 1)).astype(np.float32))

    # tril mask block [128, 128]: m[i, j] = 1 iff j >= i, bf16
    tri = (np.arange(128)[:, None] <= np.arange(128)[None, :]).astype(
        ml_dtypes.bfloat16
    )
    tri = np.ascontiguousarray(tri)

    in_maps = []
    for c in range(NCORES):
        b, g = c // 2, c % 2
        hs0 = HPC * g
        sec = np.arange(QR) + DH * hs0  # this core's rows within a section
        Wq = Wqkv[sec[perm], :]
        Wk = Wqkv[C + sec[perm], :]
        Wv = Wqkv[2 * C + sec, :]
        wqkvT = np.ascontiguousarray(
            np.concatenate([Wq, Wk, Wv], 0).T.astype(ml_dtypes.bfloat16)
        )
        woTc = np.ascontiguousarray(Wo[:, sec].T)
        xTb = np.ascontiguousarray(x[b].T.astype(ml_dtypes.bfloat16))
        in_maps.append(
            {
                "xT": xTb,
                "wqkvT": wqkvT,
                "woT": woTc,
                "cosT": cosT4,
                "sinT": sinT4,
                "maskd": tri,
            }
        )
    return in_maps


def kernel(x, cos, sin, Wqkv, Wo, _want_profile=False):
    from concourse.bass_utils import run_bass_kernel_spmd

    x = np.asarray(x, dtype=np.float32)
    cos = np.asarray(cos, dtype=np.float32)
    sin = np.asarray(sin, dtype=np.float32)
    Wqkv = np.asarray(Wqkv, dtype=np.float32)
    Wo = np.asarray(Wo, dtype=np.float32)

    nc = _get_program()
    in_maps = _host_inputs(x, cos, sin, Wqkv, Wo)
    res = run_bass_kernel_spmd(nc, in_maps, list(range(NCORES)), trace=_want_profile)
    out = np.empty((B, T, C), dtype=np.float32)
    for b in range(B):
        acc = (
            res.results[2 * b]["outT"].astype(np.float32)
            + res.results[2 * b + 1]["outT"].astype(np.float32)
        )
        out[b] = acc.T
    if _want_profile:
        return out, res
    return out


# revision 22
# speedup vs baseline: 1.0548x; 1.0077x over previous
"""Multi-head attention (RoPE + causal softmax + out-proj) on 8 TRN2 NeuronCores.

Sharding: core c handles batch b = c // 2 and head-half g = c % 2 (8 of 16
heads). Each core computes q/k/v projections for its heads, RoPE, causal
attention, and a partial transposed output projection
outT = (y_heads @ Wo_part.T).T; the host sums the two partials per batch.

Design notes (740us baseline -> ~372us):
 - x, Wqkv, q/k (post-rope), v, and exp(scores) are bf16 (measured rel err
   3.7e-3 vs the 2e-2 gate): halves input DMA + SBUF, enables FWL weight
   loads, and keeps narrow free dims at full PE rate.
 - Attention is k-major: sT = k q^T in [k:128, q:512] tiles, the 4 heads of
   a group split across two PSUM pair-tiles (sT_AB, sT_CD) so the exp
   activations (ScalarE) double-buffer with two 2-bank slots.
 - exp is causally narrowed: diagonal k-tiles only exponentiate the valid
   q-range; the 128-wide triangle strip is masked with a bf16 tril multiply
   on DVE (gpsimd op-type changes cost firmware library swaps); attn@v and
   denominator matmuls use the same narrowed range.
 - attn@v packs two heads per 512-cycle PE window via column tiling (M=64
   at tile_position (0,0)/(0,64)); the softmax denominator is a 4-way
   col-tiled ones-matmul (M=32 each) producing row-replicated sums; each
   col-tiled region starts its own PSUM accumulation group (has_written
   clears are per-region).
 - Software-pipeline skew: attn@v/denominator for k-tile kt-1 are emitted
   after scores/exp(kt), so exp-gated matmuls never head-of-line-block the
   next k-tile's scores in the strict-FIFO PE queue.
 - Out-projection chunks for q-chunk qi-1 are interleaved into qi's k-tile
   loop (dependency-free dense matmuls lengthen PE bursts past the ~3.4us
   HAM warm-up window so the PE clock reaches 2.4GHz); the final chunk's
   out-projection rotates across the idle PSUM pools for 4-deep overlap.
 - Normalization: evacuate yT psum->sbuf immediately (frees PSUM slots),
   reciprocal_approx_fast (plain reciprocal is 8 cyc/elem), DMA each
   denominator row to partition 0, partition_broadcast, multiply.
"""

import numpy as np

B, T, C, H = 4, 2048, 1024, 16
DH = C // H  # 64
NCORES = 8
HPC = H // 2  # 8 heads per core
QR = HPC * DH  # 512 rows per q/k/v section
TS = 512  # q-chunk width
NTS = T // TS  # 4
CC = C // 128  # 8 contraction chunks
NKT = T // 128  # 16 k-tiles

_CACHE = {}


def _build_program():
    import concourse.mybir as mybir
    import concourse.tile as tile
    from concourse import bacc

    f32 = mybir.dt.float32
    f32r = mybir.dt.float32r
    bf16 = mybir.dt.bfloat16
    EXP = mybir.ActivationFunctionType.Exp

    nc = bacc.Bacc(trn_type="TRN2")

    xT = nc.dram_tensor("xT", [C, T], bf16, kind="ExternalInput").ap()
    wqkvT = nc.dram_tensor("wqkvT", [C, 3 * QR], bf16, kind="ExternalInput").ap()
    woT = nc.dram_tensor("woT", [QR, C], f32, kind="ExternalInput").ap()
    cosT = nc.dram_tensor("cosT", [128, T], f32, kind="ExternalInput").ap()
    sinT = nc.dram_tensor("sinT", [128, T], f32, kind="ExternalInput").ap()
    maskd = nc.dram_tensor("maskd", [128, 128], bf16, kind="ExternalInput").ap()
    outT = nc.dram_tensor("outT", [C, T], f32, kind="ExternalOutput").ap()

    with tile.TileContext(nc) as tc:
        with tc.tile_pool(name="persist", bufs=1) as pp:
            # rope'd q/k, bf16, projection layout: key (part, grp, half):
            # rows 32*i = x-half of local head 4*grp+i
            qk = {}
            for part in range(2):
                for grp in range(2):
                    for half in range(2):
                        nm = f"qk{part}{grp}{half}"
                        qk[(part, grp, half)] = pp.tile(
                            [128, T], bf16, tag=nm, name=nm
                        )
            # v: [t-chunk 128, 8 heads * 64], bf16
            vt = [
                pp.tile([128, QR], bf16, tag=f"vt{t}", name=f"vt{t}")
                for t in range(NKT)
            ]
            maskt = pp.tile([128, 128], bf16, tag="maskt")
            onesw = pp.tile([128, 32], bf16, tag="onesw")
            ct = pp.tile([128, T], f32, tag="cos")
            st_ = pp.tile([128, T], f32, tag="sin")

            # ---------------- phase A: qkv projection + rope ----------------
            with (
                tc.tile_pool(name="wpool", bufs=1) as wp,
                tc.tile_pool(name="xpool", bufs=9) as xp,
                tc.tile_pool(name="ropetmp", bufs=8) as rt,
                tc.tile_pool(name="psA", bufs=4, space="PSUM") as psA,
            ):
                # first weight chunk + first x chunks + trig tables first,
                # then the remaining weight chunks
                wtiles = [None] * CC
                w = wp.tile([128, 3 * QR], bf16, tag="w0", name="w0")
                nc.sync.dma_start(w[:], wqkvT[0:128, :])
                wtiles[0] = w
                xts0 = []
                for cc in range(CC):
                    xt = xp.tile([128, TS], bf16, tag="xts", name=f"x0{cc}")
                    nc.sync.dma_start(xt[:], xT[128 * cc : 128 * (cc + 1), 0:TS])
                    xts0.append(xt)
                for cc in range(1, CC):
                    w = wp.tile([128, 3 * QR], bf16, tag=f"w{cc}", name=f"w{cc}")
                    nc.sync.dma_start(w[:], wqkvT[128 * cc : 128 * (cc + 1), :])
                    wtiles[cc] = w
                nc.sync.dma_start(ct[:], cosT[:])
                nc.sync.dma_start(st_[:], sinT[:])
                nc.sync.dma_start(maskt[:], maskd[:])
                nc.gpsimd.memset(onesw[:], 1.0)

                for ts in range(NTS):
                    if ts == 0:
                        xts = xts0
                    else:
                        xts = []
                        for cc in range(CC):
                            xt = xp.tile([128, TS], bf16, tag="xts", name=f"x{ts}{cc}")
                            nc.sync.dma_start(
                                xt[:],
                                xT[
                                    128 * cc : 128 * (cc + 1), TS * ts : TS * (ts + 1)
                                ],
                            )
                            xts.append(xt)

                    # q/k projections -> rope (mul on DVE, add/sub on
                    # GPSIMD, writing bf16 into the persistent qk tiles)
                    for part in range(2):  # 0=q, 1=k
                        for grp in range(2):  # local heads 4*grp .. 4*grp+3
                            ptiles = []
                            for half in range(2):  # x1, x2
                                p = psA.tile(
                                    [128, TS], f32, tag="proj",
                                    name=f"p{ts}{part}{grp}{half}",
                                )
                                col0 = QR * part + 256 * grp + 128 * half
                                for cc in range(CC):
                                    nc.tensor.matmul(
                                        p[:],
                                        wtiles[cc][:, col0 : col0 + 128],
                                        xts[cc][:],
                                        start=(cc == 0),
                                        stop=(cc == CC - 1),
                                    )
                                ptiles.append(p)
                            x1p, x2p = ptiles
                            csl = ct[:, TS * ts : TS * (ts + 1)]
                            ssl = st_[:, TS * ts : TS * (ts + 1)]
                            o1 = qk[(part, grp, 0)][:, TS * ts : TS * (ts + 1)]
                            o2 = qk[(part, grp, 1)][:, TS * ts : TS * (ts + 1)]
                            t1 = rt.tile([128, TS], f32, tag="rt", name=f"t1{ts}{part}{grp}")
                            t2 = rt.tile([128, TS], f32, tag="rt", name=f"t2{ts}{part}{grp}")
                            nc.vector.tensor_mul(t1[:], x1p[:], csl)
                            nc.vector.tensor_mul(t2[:], x2p[:], ssl)
                            nc.gpsimd.tensor_sub(o1, t1[:], t2[:])
                            t3 = rt.tile([128, TS], f32, tag="rt", name=f"t3{ts}{part}{grp}")
                            t4 = rt.tile([128, TS], f32, tag="rt", name=f"t4{ts}{part}{grp}")
                            nc.vector.tensor_mul(t3[:], x1p[:], ssl)
                            nc.vector.tensor_mul(t4[:], x2p[:], csl)
                            nc.gpsimd.tensor_add(o2, t3[:], t4[:])

                    # v projection; psum -> bf16 copy on ScalarE (idle here)
                    for tr4 in range(4):
                        t = 4 * ts + tr4
                        p = psA.tile([128, QR], f32, tag="proj", name=f"pv{ts}{tr4}")
                        for cc in range(CC):
                            nc.tensor.matmul(
                                p[:],
                                xts[cc][:, 128 * tr4 : 128 * (tr4 + 1)],
                                wtiles[cc][:, 2 * QR : 3 * QR],
                                start=(cc == 0),
                                stop=(cc == CC - 1),
                            )
                        nc.scalar.copy(vt[t][:], p[:])

            # ---------------- phase B: attention + out-proj ----------------
            with (
                tc.tile_pool(name="wopool", bufs=1) as wop,
                tc.tile_pool(name="epool", bufs=6) as ep,
                tc.tile_pool(name="rcpool", bufs=4) as rp,
                tc.tile_pool(name="bcpool", bufs=4) as bp,
                tc.tile_pool(name="ynpool", bufs=8) as yp,
                tc.tile_pool(name="ostage", bufs=4) as osp,
                tc.tile_pool(name="psS", bufs=2, space="PSUM") as psS,
                tc.tile_pool(name="psY", bufs=2, space="PSUM") as psY,
                tc.tile_pool(name="psD", bufs=1, space="PSUM") as psD,
                tc.tile_pool(name="psW", bufs=1, space="PSUM") as psW,
            ):
                wot = []
                for j in range(4):
                    w = wop.tile([128, C], f32r, tag=f"wo{j}", name=f"wo{j}")
                    nc.sync.dma_start(
                        w[:], woT[128 * j : 128 * (j + 1), :].bitcast(f32r)
                    )
                    wot.append(w)

                def outproj_chunk(qj, co, ynormq, pool=None, tag="op"):
                    """One 128-channel out-projection chunk for q-chunk qj:
                    4 dense matmuls + evacuate + store. Interleaved into the
                    next chunk's attention to keep the PE bursts long."""
                    p = (pool or psW).tile([128, TS], f32, tag=tag, name=f"o{qj}{co}")
                    for j in range(4):
                        nc.tensor.matmul(
                            p[:],
                            wot[j][:, 128 * co : 128 * (co + 1)],
                            ynormq[j][:],
                            start=(j == 0),
                            stop=(j == 3),
                        )
                    o = osp.tile([128, TS], f32, tag="os", name=f"os{qj}{co}")
                    nc.vector.tensor_copy(o[:], p[:])
                    nc.sync.dma_start(
                        outT[128 * co : 128 * (co + 1), TS * qj : TS * (qj + 1)],
                        o[:],
                    )

                prev_ynorm = None
                for qi in range(NTS):
                    q0 = TS * qi
                    nkt = 4 * (qi + 1)
                    # schedule of leftover out-proj chunks from qi-1,
                    # spread across this chunk's k-tile iterations
                    op_sched = {}
                    if prev_ynorm is not None:
                        stride = max(1, (2 * nkt) // 8)
                        for co in range(8):
                            op_sched.setdefault(co * stride, []).append(co)
                    opk = 0  # global kt counter across both g4 groups
                    ynorm = {}
                    for g4 in range(2):
                        yTp = [
                            psY.tile([128, TS], f32, tag="yT", name=f"yT{g4}_{qi}_{p}")
                            for p in range(2)
                        ]
                        dn = psD.tile([128, TS], f32, tag="dn", name=f"dn{g4}_{qi}")

                        def attnv_denom(kt, ets):
                            """attn@v + denominator matmuls for k-tile kt
                            (emitted one iteration late: software-pipeline
                            skew so these exp-gated matmuls never block
                            the next k-tile's scores in the PE queue)."""
                            r = kt - 4 * qi
                            off = 128 * r if r >= 0 else 0
                            for pr in range(2):
                                for lh in range(2):
                                    h = 4 * g4 + 2 * pr + lh
                                    nc.tensor.matmul(
                                        yTp[pr][64 * lh : 64 * (lh + 1), off:TS],
                                        vt[kt][:, DH * h : DH * (h + 1)],
                                        ets[pr][:, TS * lh + off : TS * (lh + 1)],
                                        start=(kt == 0),
                                        stop=(kt == nkt - 1),
                                        tile_position=(0, 64 * lh),
                                    )
                            for pr in range(2):
                                for lh in range(2):
                                    j32 = 32 * (2 * pr + lh)
                                    nc.tensor.matmul(
                                        dn[j32 : j32 + 32, off:TS],
                                        onesw[:, 0:32],
                                        ets[pr][:, TS * lh + off : TS * (lh + 1)],
                                        start=(kt == 0),
                                        stop=(kt == nkt - 1),
                                        tile_position=(0, j32),
                                    )

                        prev = None
                        for kt in range(nkt):
                            k0 = 128 * kt
                            r = kt - 4 * qi
                            off = 128 * r if r >= 0 else 0
                            sp = [
                                psS.tile(
                                    [128, 2 * TS], f32, tag="sT",
                                    name=f"sT{g4}_{qi}_{kt}_{p}",
                                )
                                for p in range(2)
                            ]
                            # scores: 4 heads on 4 row groups, 2 halves
                            # accumulating per head; half-outer emission so
                            # the 4 start-matmuls run concurrently and the
                            # accumulating round pipelines behind them.
                            for half in range(2):
                                for pr in range(2):
                                    for lh in range(2):
                                        rb = 32 * (2 * pr + lh)
                                        nc.tensor.matmul(
                                            sp[pr][:, TS * lh + off : TS * (lh + 1)],
                                            qk[(1, g4, half)][
                                                rb : rb + 32, k0 : k0 + 128
                                            ],
                                            qk[(0, g4, half)][
                                                rb : rb + 32, q0 + off : q0 + TS
                                            ],
                                            start=(half == 0),
                                            stop=(half == 1),
                                            tile_position=(rb, 0),
                                        )
                            ets = []
                            for pr in range(2):
                                eT = ep.tile(
                                    [128, 2 * TS], bf16, tag="eT",
                                    name=f"eT{g4}_{qi}_{kt}_{pr}",
                                )
                                if off == 0:
                                    nc.scalar.activation(
                                        eT[:], sp[pr][:], EXP, scale=0.125
                                    )
                                else:
                                    e3 = eT.rearrange("p (h q) -> p h q", h=2)
                                    s3 = sp[pr].rearrange("p (h q) -> p h q", h=2)
                                    nc.scalar.activation(
                                        e3[:, :, off:TS],
                                        s3[:, :, off:TS],
                                        EXP,
                                        scale=0.125,
                                    )
                                if r >= 0:
                                    # mask the 128-wide triangle strip
                                    e3 = eT.rearrange("p (h q) -> p h q", h=2)
                                    nc.vector.tensor_mul(
                                        e3[:, :, off : off + 128],
                                        e3[:, :, off : off + 128],
                                        maskt[:].unsqueeze(1).broadcast_to(
                                            [128, 2, 128]
                                        ),
                                    )
                                ets.append(eT)
                            if prev is not None:
                                attnv_denom(*prev)
                            for co in op_sched.get(opk, ()):
                                outproj_chunk(qi - 1, co, prev_ynorm)
                            opk += 1
                            prev = (kt, ets)
                        attnv_denom(*prev)
                        # normalization: evacuate yT psum -> sbuf (frees
                        # psY fast), fast-approx reciprocal, stage each
                        # head's denominator row to partition 0 (DMA),
                        # partition_broadcast, multiply.
                        ysb = []
                        for pr in range(2):
                            ys = rp.tile(
                                [128, TS], f32, tag="ysb", name=f"ys{g4}_{qi}_{pr}"
                            )
                            nc.vector.tensor_copy(ys[:], yTp[pr][:])
                            ysb.append(ys)
                        rcp = rp.tile([128, TS], f32, tag="rcp", name=f"rcp{g4}_{qi}")
                        nc.vector.reciprocal_approx_fast(rcp[:], dn[:])
                        for pr in range(2):
                            yn = yp.tile(
                                [128, TS], f32r, tag="yn", name=f"yn{g4}_{qi}_{pr}"
                            )
                            for lh in range(2):
                                j32 = 32 * (2 * pr + lh)
                                stg = rp.tile(
                                    [1, TS], f32, tag="stg", name=f"sg{g4}{qi}{pr}{lh}"
                                )
                                nc.sync.dma_start(stg[:], rcp[j32 : j32 + 1, :])
                                bc = bp.tile(
                                    [128, TS], f32, tag="bc", name=f"bc{g4}{qi}{pr}{lh}"
                                )
                                nc.gpsimd.partition_broadcast(bc[:], stg[:])
                                nc.vector.tensor_mul(
                                    yn[64 * lh : 64 * (lh + 1), :],
                                    ysb[pr][64 * lh : 64 * (lh + 1), :],
                                    bc[64 * lh : 64 * (lh + 1), :],
                                )
                            ynorm[2 * g4 + pr] = yn
                    # any leftover chunks of qi-1 not yet emitted
                    if prev_ynorm is not None:
                        for k2 in sorted(op_sched):
                            if k2 >= opk:
                                for co in op_sched[k2]:
                                    outproj_chunk(qi - 1, co, prev_ynorm)
                    prev_ynorm = ynorm
                # final q-chunk's out-projection (pipeline tail):
                # rotate across the now-idle PSUM pools for 4-deep overlap
                tail_pools = [(psW, "op"), (psD, "dn"), (psY, "yT"), (psY, "yT")]
                for co in range(8):
                    pool, tag = tail_pools[co % 4]
                    outproj_chunk(NTS - 1, co, prev_ynorm, pool=pool, tag=tag)

    nc.compile()
    return nc


def _get_program():
    if "nc" not in _CACHE:
        _CACHE["nc"] = _build_program()
    return _CACHE["nc"]


def _host_inputs(x, cos, sin, Wqkv, Wo):
    """Build the 8 per-core input maps."""
    import ml_dtypes

    # permutation of one head-section's 512 rows (head-relative):
    # row-tile layout [x1 h0-3 | x2 h0-3 | x1 h4-7 | x2 h4-7], 32 rows/block
    perm = []
    for grp in range(2):
        for half in range(2):
            for lh in range(4 * grp, 4 * grp + 4):
                for jj in range(32):
                    perm.append(64 * lh + 2 * jj + half)
    perm = np.asarray(perm)

    cosT4 = np.ascontiguousarray(np.tile(cos.T, (4, 1)).astype(np.float32))
    sinT4 = np.ascontiguousarray(np.tile(sin.T, (4, 1)).astype(np.float32))

    # tril mask block [128, 128]: m[i, j] = 1 iff j >= i, bf16
    tri = (np.arange(128)[:, None] <= np.arange(128)[None, :]).astype(
        ml_dtypes.bfloat16
    )
    tri = np.ascontiguousarray(tri)

    in_maps = []
    for c in range(NCORES):
        b, g = c // 2, c % 2
        hs0 = HPC * g
        sec = np.arange(QR) + DH * hs0  # this core's rows within a section
        Wq = Wqkv[sec[perm], :]
        Wk = Wqkv[C + sec[perm], :]
        Wv = Wqkv[2 * C + sec, :]
        wqkvT = np.ascontiguousarray(
            np.concatenate([Wq, Wk, Wv], 0).T.astype(ml_dtypes.bfloat16)
        )
        woTc = np.ascontiguousarray(Wo[:, sec].T)
        xTb = np.ascontiguousarray(x[b].T.astype(ml_dtypes.bfloat16))
        in_maps.append(
            {
                "xT": xTb,
                "wqkvT": wqkvT,
                "woT": woTc,
                "cosT": cosT4,
                "sinT": sinT4,
                "maskd": tri,
            }
        )
    return in_maps


def kernel(x, cos, sin, Wqkv, Wo, _want_profile=False):
    from concourse.bass_utils import run_bass_kernel_spmd

    x = np.asarray(x, dtype=np.float32)
    cos = np.asarray(cos, dtype=np.float32)
    sin = np.asarray(sin, dtype=np.float32)
    Wqkv = np.asarray(Wqkv, dtype=np.float32)
    Wo = np.asarray(Wo, dtype=np.float32)

    nc = _get_program()
    in_maps = _host_inputs(x, cos, sin, Wqkv, Wo)
    res = run_bass_kernel_spmd(nc, in_maps, list(range(NCORES)), trace=_want_profile)
    out = np.empty((B, T, C), dtype=np.float32)
    for b in range(B):
        acc = (
            res.results[2 * b]["outT"].astype(np.float32)
            + res.results[2 * b + 1]["outT"].astype(np.float32)
        )
        out[b] = acc.T
    if _want_profile:
        return out, res
    return out


# revision 23
# speedup vs baseline: 1.0572x; 1.0022x over previous
"""Multi-head attention (RoPE + causal softmax + out-proj) on 8 TRN2 NeuronCores.

Sharding: core c handles batch b = c // 2 and head-half g = c % 2 (8 of 16
heads). Each core computes q/k/v projections for its heads, RoPE, causal
attention, and a partial transposed output projection
outT = (y_heads @ Wo_part.T).T; the host sums the two partials per batch.

Design notes (740us baseline -> ~372us):
 - x, Wqkv, q/k (post-rope), v, and exp(scores) are bf16 (measured rel err
   3.7e-3 vs the 2e-2 gate): halves input DMA + SBUF, enables FWL weight
   loads, and keeps narrow free dims at full PE rate.
 - Attention is k-major: sT = k q^T in [k:128, q:512] tiles, the 4 heads of
   a group split across two PSUM pair-tiles (sT_AB, sT_CD) so the exp
   activations (ScalarE) double-buffer with two 2-bank slots.
 - exp is causally narrowed: diagonal k-tiles only exponentiate the valid
   q-range; the 128-wide triangle strip is masked with a bf16 tril multiply
   on DVE (gpsimd op-type changes cost firmware library swaps); attn@v and
   denominator matmuls use the same narrowed range.
 - attn@v packs two heads per 512-cycle PE window via column tiling (M=64
   at tile_position (0,0)/(0,64)); the softmax denominator is a 4-way
   col-tiled ones-matmul (M=32 each) producing row-replicated sums; each
   col-tiled region starts its own PSUM accumulation group (has_written
   clears are per-region).
 - Software-pipeline skew: attn@v/denominator for k-tile kt-1 are emitted
   after scores/exp(kt), so exp-gated matmuls never head-of-line-block the
   next k-tile's scores in the strict-FIFO PE queue.
 - Out-projection chunks for q-chunk qi-1 are interleaved into qi's k-tile
   loop (dependency-free dense matmuls lengthen PE bursts past the ~3.4us
   HAM warm-up window so the PE clock reaches 2.4GHz); the final chunk's
   out-projection rotates across the idle PSUM pools for 4-deep overlap.
 - Normalization: evacuate yT psum->sbuf immediately (frees PSUM slots),
   reciprocal_approx_fast (plain reciprocal is 8 cyc/elem), DMA each
   denominator row to partition 0, partition_broadcast, multiply.
"""

import numpy as np

B, T, C, H = 4, 2048, 1024, 16
DH = C // H  # 64
NCORES = 8
HPC = H // 2  # 8 heads per core
QR = HPC * DH  # 512 rows per q/k/v section
TS = 512  # q-chunk width
NTS = T // TS  # 4
CC = C // 128  # 8 contraction chunks
NKT = T // 128  # 16 k-tiles

_CACHE = {}


def _build_program():
    import concourse.mybir as mybir
    import concourse.tile as tile
    from concourse import bacc

    f32 = mybir.dt.float32
    f32r = mybir.dt.float32r
    bf16 = mybir.dt.bfloat16
    EXP = mybir.ActivationFunctionType.Exp

    nc = bacc.Bacc(trn_type="TRN2")

    xT = nc.dram_tensor("xT", [C, T], bf16, kind="ExternalInput").ap()
    wqkvT = nc.dram_tensor("wqkvT", [C, 3 * QR], bf16, kind="ExternalInput").ap()
    woT = nc.dram_tensor("woT", [QR, C], f32, kind="ExternalInput").ap()
    cosT = nc.dram_tensor("cosT", [128, T], bf16, kind="ExternalInput").ap()
    sinT = nc.dram_tensor("sinT", [128, T], bf16, kind="ExternalInput").ap()
    maskd = nc.dram_tensor("maskd", [128, 128], bf16, kind="ExternalInput").ap()
    outT = nc.dram_tensor("outT", [C, T], f32, kind="ExternalOutput").ap()

    with tile.TileContext(nc) as tc:
        with tc.tile_pool(name="persist", bufs=1) as pp:
            # rope'd q/k, bf16, projection layout: key (part, grp, half):
            # rows 32*i = x-half of local head 4*grp+i
            qk = {}
            for part in range(2):
                for grp in range(2):
                    for half in range(2):
                        nm = f"qk{part}{grp}{half}"
                        qk[(part, grp, half)] = pp.tile(
                            [128, T], bf16, tag=nm, name=nm
                        )
            # v: [t-chunk 128, 8 heads * 64], bf16
            vt = [
                pp.tile([128, QR], bf16, tag=f"vt{t}", name=f"vt{t}")
                for t in range(NKT)
            ]
            maskt = pp.tile([128, 128], bf16, tag="maskt")
            onesw = pp.tile([128, 32], bf16, tag="onesw")
            ct = pp.tile([128, T], bf16, tag="cos")
            st_ = pp.tile([128, T], bf16, tag="sin")

            # ---------------- phase A: qkv projection + rope ----------------
            with (
                tc.tile_pool(name="wpool", bufs=1) as wp,
                tc.tile_pool(name="xpool", bufs=9) as xp,
                tc.tile_pool(name="ropetmp", bufs=8) as rt,
                tc.tile_pool(name="psA", bufs=4, space="PSUM") as psA,
            ):
                # first weight chunk + first x chunks + trig tables first,
                # then the remaining weight chunks
                wtiles = [None] * CC
                w = wp.tile([128, 3 * QR], bf16, tag="w0", name="w0")
                nc.sync.dma_start(w[:], wqkvT[0:128, :])
                wtiles[0] = w
                xts0 = []
                for cc in range(CC):
                    xt = xp.tile([128, TS], bf16, tag="xts", name=f"x0{cc}")
                    nc.sync.dma_start(xt[:], xT[128 * cc : 128 * (cc + 1), 0:TS])
                    xts0.append(xt)
                for cc in range(1, CC):
                    w = wp.tile([128, 3 * QR], bf16, tag=f"w{cc}", name=f"w{cc}")
                    nc.sync.dma_start(w[:], wqkvT[128 * cc : 128 * (cc + 1), :])
                    wtiles[cc] = w
                nc.sync.dma_start(ct[:], cosT[:])
                nc.sync.dma_start(st_[:], sinT[:])
                nc.sync.dma_start(maskt[:], maskd[:])
                nc.gpsimd.memset(onesw[:], 1.0)

                for ts in range(NTS):
                    if ts == 0:
                        xts = xts0
                    else:
                        xts = []
                        for cc in range(CC):
                            xt = xp.tile([128, TS], bf16, tag="xts", name=f"x{ts}{cc}")
                            nc.sync.dma_start(
                                xt[:],
                                xT[
                                    128 * cc : 128 * (cc + 1), TS * ts : TS * (ts + 1)
                                ],
                            )
                            xts.append(xt)

                    # q/k projections -> rope (mul on DVE, add/sub on
                    # GPSIMD, writing bf16 into the persistent qk tiles)
                    for part in range(2):  # 0=q, 1=k
                        for grp in range(2):  # local heads 4*grp .. 4*grp+3
                            ptiles = []
                            for half in range(2):  # x1, x2
                                p = psA.tile(
                                    [128, TS], f32, tag="proj",
                                    name=f"p{ts}{part}{grp}{half}",
                                )
                                col0 = QR * part + 256 * grp + 128 * half
                                for cc in range(CC):
                                    nc.tensor.matmul(
                                        p[:],
                                        wtiles[cc][:, col0 : col0 + 128],
                                        xts[cc][:],
                                        start=(cc == 0),
                                        stop=(cc == CC - 1),
                                    )
                                ptiles.append(p)
                            x1p, x2p = ptiles
                            # evacuate psum -> bf16 sbuf on ScalarE (idle
                            # here): frees the psum slot ~2x sooner and
                            # puts the rope in DVE's 2x packed-bf16 mode
                            x1b = rt.tile([128, TS], bf16, tag="xh", name=f"x1b{ts}{part}{grp}")
                            x2b = rt.tile([128, TS], bf16, tag="xh", name=f"x2b{ts}{part}{grp}")
                            nc.scalar.copy(x1b[:], x1p[:])
                            nc.scalar.copy(x2b[:], x2p[:])
                            csl = ct[:, TS * ts : TS * (ts + 1)]
                            ssl = st_[:, TS * ts : TS * (ts + 1)]
                            o1 = qk[(part, grp, 0)][:, TS * ts : TS * (ts + 1)]
                            o2 = qk[(part, grp, 1)][:, TS * ts : TS * (ts + 1)]
                            t1 = rt.tile([128, TS], bf16, tag="rt", name=f"t1{ts}{part}{grp}")
                            t2 = rt.tile([128, TS], bf16, tag="rt", name=f"t2{ts}{part}{grp}")
                            nc.vector.tensor_mul(t1[:], x1b[:], csl)
                            nc.vector.tensor_mul(t2[:], x2b[:], ssl)
                            nc.gpsimd.tensor_sub(o1, t1[:], t2[:])
                            t3 = rt.tile([128, TS], bf16, tag="rt", name=f"t3{ts}{part}{grp}")
                            t4 = rt.tile([128, TS], bf16, tag="rt", name=f"t4{ts}{part}{grp}")
                            nc.vector.tensor_mul(t3[:], x1b[:], ssl)
                            nc.vector.tensor_mul(t4[:], x2b[:], csl)
                            nc.gpsimd.tensor_add(o2, t3[:], t4[:])

                    # v projection; psum -> bf16 copy on ScalarE (idle here)
                    for tr4 in range(4):
                        t = 4 * ts + tr4
                        p = psA.tile([128, QR], f32, tag="proj", name=f"pv{ts}{tr4}")
                        for cc in range(CC):
                            nc.tensor.matmul(
                                p[:],
                                xts[cc][:, 128 * tr4 : 128 * (tr4 + 1)],
                                wtiles[cc][:, 2 * QR : 3 * QR],
                                start=(cc == 0),
                                stop=(cc == CC - 1),
                            )
                        nc.scalar.copy(vt[t][:], p[:])

            # ---------------- phase B: attention + out-proj ----------------
            with (
                tc.tile_pool(name="wopool", bufs=1) as wop,
                tc.tile_pool(name="epool", bufs=6) as ep,
                tc.tile_pool(name="rcpool", bufs=4) as rp,
                tc.tile_pool(name="bcpool", bufs=4) as bp,
                tc.tile_pool(name="ynpool", bufs=8) as yp,
                tc.tile_pool(name="ostage", bufs=4) as osp,
                tc.tile_pool(name="psS", bufs=2, space="PSUM") as psS,
                tc.tile_pool(name="psY", bufs=2, space="PSUM") as psY,
                tc.tile_pool(name="psD", bufs=1, space="PSUM") as psD,
                tc.tile_pool(name="psW", bufs=1, space="PSUM") as psW,
            ):
                wot = []
                for j in range(4):
                    w = wop.tile([128, C], f32r, tag=f"wo{j}", name=f"wo{j}")
                    nc.sync.dma_start(
                        w[:], woT[128 * j : 128 * (j + 1), :].bitcast(f32r)
                    )
                    wot.append(w)

                def outproj_chunk(qj, co, ynormq, pool=None, tag="op"):
                    """One 128-channel out-projection chunk for q-chunk qj:
                    4 dense matmuls + evacuate + store. Interleaved into the
                    next chunk's attention to keep the PE bursts long."""
                    p = (pool or psW).tile([128, TS], f32, tag=tag, name=f"o{qj}{co}")
                    for j in range(4):
                        nc.tensor.matmul(
                            p[:],
                            wot[j][:, 128 * co : 128 * (co + 1)],
                            ynormq[j][:],
                            start=(j == 0),
                            stop=(j == 3),
                        )
                    o = osp.tile([128, TS], f32, tag="os", name=f"os{qj}{co}")
                    nc.vector.tensor_copy(o[:], p[:])
                    nc.sync.dma_start(
                        outT[128 * co : 128 * (co + 1), TS * qj : TS * (qj + 1)],
                        o[:],
                    )

                prev_ynorm = None
                for qi in range(NTS):
                    q0 = TS * qi
                    nkt = 4 * (qi + 1)
                    # schedule of leftover out-proj chunks from qi-1,
                    # spread across this chunk's k-tile iterations
                    op_sched = {}
                    if prev_ynorm is not None:
                        stride = max(1, (2 * nkt) // 8)
                        for co in range(8):
                            op_sched.setdefault(co * stride, []).append(co)
                    opk = 0  # global kt counter across both g4 groups
                    ynorm = {}
                    for g4 in range(2):
                        yTp = [
                            psY.tile([128, TS], f32, tag="yT", name=f"yT{g4}_{qi}_{p}")
                            for p in range(2)
                        ]
                        dn = psD.tile([128, TS], f32, tag="dn", name=f"dn{g4}_{qi}")

                        def attnv_denom(kt, ets):
                            """attn@v + denominator matmuls for k-tile kt
                            (emitted one iteration late: software-pipeline
                            skew so these exp-gated matmuls never block
                            the next k-tile's scores in the PE queue)."""
                            r = kt - 4 * qi
                            off = 128 * r if r >= 0 else 0
                            for pr in range(2):
                                for lh in range(2):
                                    h = 4 * g4 + 2 * pr + lh
                                    nc.tensor.matmul(
                                        yTp[pr][64 * lh : 64 * (lh + 1), off:TS],
                                        vt[kt][:, DH * h : DH * (h + 1)],
                                        ets[pr][:, TS * lh + off : TS * (lh + 1)],
                                        start=(kt == 0),
                                        stop=(kt == nkt - 1),
                                        tile_position=(0, 64 * lh),
                                    )
                            for pr in range(2):
                                for lh in range(2):
                                    j32 = 32 * (2 * pr + lh)
                                    nc.tensor.matmul(
                                        dn[j32 : j32 + 32, off:TS],
                                        onesw[:, 0:32],
                                        ets[pr][:, TS * lh + off : TS * (lh + 1)],
                                        start=(kt == 0),
                                        stop=(kt == nkt - 1),
                                        tile_position=(0, j32),
                                    )

                        prev = None
                        for kt in range(nkt):
                            k0 = 128 * kt
                            r = kt - 4 * qi
                            off = 128 * r if r >= 0 else 0
                            sp = [
                                psS.tile(
                                    [128, 2 * TS], f32, tag="sT",
                                    name=f"sT{g4}_{qi}_{kt}_{p}",
                                )
                                for p in range(2)
                            ]
                            # scores: 4 heads on 4 row groups, 2 halves
                            # accumulating per head; half-outer emission so
                            # the 4 start-matmuls run concurrently and the
                            # accumulating round pipelines behind them.
                            for half in range(2):
                                for pr in range(2):
                                    for lh in range(2):
                                        rb = 32 * (2 * pr + lh)
                                        nc.tensor.matmul(
                                            sp[pr][:, TS * lh + off : TS * (lh + 1)],
                                            qk[(1, g4, half)][
                                                rb : rb + 32, k0 : k0 + 128
                                            ],
                                            qk[(0, g4, half)][
                                                rb : rb + 32, q0 + off : q0 + TS
                                            ],
                                            start=(half == 0),
                                            stop=(half == 1),
                                            tile_position=(rb, 0),
                                        )
                            ets = []
                            for pr in range(2):
                                eT = ep.tile(
                                    [128, 2 * TS], bf16, tag="eT",
                                    name=f"eT{g4}_{qi}_{kt}_{pr}",
                                )
                                if off == 0:
                                    nc.scalar.activation(
                                        eT[:], sp[pr][:], EXP, scale=0.125
                                    )
                                else:
                                    e3 = eT.rearrange("p (h q) -> p h q", h=2)
                                    s3 = sp[pr].rearrange("p (h q) -> p h q", h=2)
                                    nc.scalar.activation(
                                        e3[:, :, off:TS],
                                        s3[:, :, off:TS],
                                        EXP,
                                        scale=0.125,
                                    )
                                if r >= 0:
                                    # mask the 128-wide triangle strip
                                    e3 = eT.rearrange("p (h q) -> p h q", h=2)
                                    nc.vector.tensor_mul(
                                        e3[:, :, off : off + 128],
                                        e3[:, :, off : off + 128],
                                        maskt[:].unsqueeze(1).broadcast_to(
                                            [128, 2, 128]
                                        ),
                                    )
                                ets.append(eT)
                            if prev is not None:
                                attnv_denom(*prev)
                            for co in op_sched.get(opk, ()):
                                outproj_chunk(qi - 1, co, prev_ynorm)
                            opk += 1
                            prev = (kt, ets)
                        attnv_denom(*prev)
                        # normalization: evacuate yT psum -> sbuf (frees
                        # psY fast), fast-approx reciprocal, stage each
                        # head's denominator row to partition 0 (DMA),
                        # partition_broadcast, multiply.
                        ysb = []
                        for pr in range(2):
                            ys = rp.tile(
                                [128, TS], f32, tag="ysb", name=f"ys{g4}_{qi}_{pr}"
                            )
                            nc.vector.tensor_copy(ys[:], yTp[pr][:])
                            ysb.append(ys)
                        rcp = rp.tile([128, TS], f32, tag="rcp", name=f"rcp{g4}_{qi}")
                        nc.vector.reciprocal_approx_fast(rcp[:], dn[:])
                        for pr in range(2):
                            yn = yp.tile(
                                [128, TS], f32r, tag="yn", name=f"yn{g4}_{qi}_{pr}"
                            )
                            for lh in range(2):
                                j32 = 32 * (2 * pr + lh)
                                stg = rp.tile(
                                    [1, TS], f32, tag="stg", name=f"sg{g4}{qi}{pr}{lh}"
                                )
                                nc.sync.dma_start(stg[:], rcp[j32 : j32 + 1, :])
                                bc = bp.tile(
                                    [128, TS], f32, tag="bc", name=f"bc{g4}{qi}{pr}{lh}"
                                )
                                nc.gpsimd.partition_broadcast(bc[:], stg[:])
                                nc.vector.tensor_mul(
                                    yn[64 * lh : 64 * (lh + 1), :],
                                    ysb[pr][64 * lh : 64 * (lh + 1), :],
                                    bc[64 * lh : 64 * (lh + 1), :],
                                )
                            ynorm[2 * g4 + pr] = yn
                    # any leftover chunks of qi-1 not yet emitted
                    if prev_ynorm is not None:
                        for k2 in sorted(op_sched):
                            if k2 >= opk:
                                for co in op_sched[k2]:
                                    outproj_chunk(qi - 1, co, prev_ynorm)
                    prev_ynorm = ynorm
                # final q-chunk's out-projection (pipeline tail):
                # rotate across the now-idle PSUM pools for 4-deep overlap
                tail_pools = [(psW, "op"), (psD, "dn"), (psY, "yT"), (psY, "yT")]
                for co in range(8):
                    pool, tag = tail_pools[co % 4]
                    outproj_chunk(NTS - 1, co, prev_ynorm, pool=pool, tag=tag)

    nc.compile()
    return nc


def _get_program():
    if "nc" not in _CACHE:
        _CACHE["nc"] = _build_program()
    return _CACHE["nc"]


def _host_inputs(x, cos, sin, Wqkv, Wo):
    """Build the 8 per-core input maps."""
    import ml_dtypes

    # permutation of one head-section's 512 rows (head-relative):
    # row-tile layout [x1 h0-3 | x2 h0-3 | x1 h4-7 | x2 h4-7], 32 rows/block
    perm = []
    for grp in range(2):
        for half in range(2):
            for lh in range(4 * grp, 4 * grp + 4):
                for jj in range(32):
                    perm.append(64 * lh + 2 * jj + half)
    perm = np.asarray(perm)

    cosT4 = np.ascontiguousarray(np.tile(cos.T, (4, 1)).astype(ml_dtypes.bfloat16))
    sinT4 = np.ascontiguousarray(np.tile(sin.T, (4, 1)).astype(ml_dtypes.bfloat16))

    # tril mask block [128, 128]: m[i, j] = 1 iff j >= i, bf16
    tri = (np.arange(128)[:, None] <= np.arange(128)[None, :]).astype(
        ml_dtypes.bfloat16
    )
    tri = np.ascontiguousarray(tri)

    in_maps = []
    for c in range(NCORES):
        b, g = c // 2, c % 2
        hs0 = HPC * g
        sec = np.arange(QR) + DH * hs0  # this core's rows within a section
        Wq = Wqkv[sec[perm], :]
        Wk = Wqkv[C + sec[perm], :]
        Wv = Wqkv[2 * C + sec, :]
        wqkvT = np.ascontiguousarray(
            np.concatenate([Wq, Wk, Wv], 0).T.astype(ml_dtypes.bfloat16)
        )
        woTc = np.ascontiguousarray(Wo[:, sec].T)
        xTb = np.ascontiguousarray(x[b].T.astype(ml_dtypes.bfloat16))
        in_maps.append(
            {
                "xT": xTb,
                "wqkvT": wqkvT,
                "woT": woTc,
                "cosT": cosT4,
                "sinT": sinT4,
                "maskd": tri,
            }
        )
    return in_maps


def kernel(x, cos, sin, Wqkv, Wo, _want_profile=False):
    from concourse.bass_utils import run_bass_kernel_spmd

    x = np.asarray(x, dtype=np.float32)
    cos = np.asarray(cos, dtype=np.float32)
    sin = np.asarray(sin, dtype=np.float32)
    Wqkv = np.asarray(Wqkv, dtype=np.float32)
    Wo = np.asarray(Wo, dtype=np.float32)

    nc = _get_program()
    in_maps = _host_inputs(x, cos, sin, Wqkv, Wo)
    res = run_bass_kernel_spmd(nc, in_maps, list(range(NCORES)), trace=_want_profile)
    out = np.empty((B, T, C), dtype=np.float32)
    for b in range(B):
        acc = (
            res.results[2 * b]["outT"].astype(np.float32)
            + res.results[2 * b + 1]["outT"].astype(np.float32)
        )
        out[b] = acc.T
    if _want_profile:
        return out, res
    return out
